# revision 1
# baseline (speedup 1.0000x reference)
"""Trainium2 Bass kernel for nn_Conv2DLinalgRMSNorm (two launches, bf16).

Math: out = RMSNormEps(x @ (sum_l conv_w[l])^T / 20) * norm_w
  where RMSNormEps(v) = v / sqrt(sum_h v^2 + eps*H) * sqrt(H)
The 1/20 folds into the norm bias: with y = x @ Wsum^T,
  out = y * sqrt(H) / sqrt(sum y^2 + NL^2*eps*H) * norm_w.

Strategy (8 NeuronCores):
  All dtype conversion / transposition happens on the host (free).
  Launch 1 (weight prep): core c owns output-channel rows [128c,128c+128)
    of the 20 conv weights, pre-arranged by the host as
    [8 hc][128 h][10][2][128 o] bf16 so the 20-layer sum is a
    5-instruction strided DVE pairwise tree per h-chunk whose result
    [128 h, 128 o] is already transposed for the launch-2 GEMM.
    Output: piece [8 hc, 128 h, 128 o] bf16 (0.25 MiB).
  Host assembles the 8 pieces into WT [128 p][8 hc][8 c][128 o] bf16.
  Launch 2 (token-parallel GEMM + norm): core c takes 1024 tokens,
    x pre-transposed on host to [4 tg][128 h][8 hc][256 t] bf16.
    GEMM in bf16 (1 cyc/row), f32 PSUM, no on-device transposes.
    RMSNorm fused: DVE copy (bf16 round) -> DVE ssq via
    scalar_tensor_tensor accum -> +eps bias -> reciprocal -> ACT sqrt
    (only ACT function => single table load) -> DVE scale*norm_w.
    y written bf16, upcast on host.
"""
import numpy as np
import ml_dtypes

import concourse.bass as bass
import concourse.mybir as mybir
from concourse.tile import TileContext
from concourse import bass_utils

dt = mybir.dt
P = 128
H = 1024
NL = 20
B, S = 2, 4096
TOK = B * S            # 8192
NCORES = 8
TPC = TOK // NCORES    # 1024 tokens per core
NCH = 8                # h chunks of 128
NTG = 4                # token groups of 256
TGW = TPC // NTG       # 256
NT = TPC // P          # 8 token tiles
EPS = 1e-6
SSQ_BIAS = float(NL * NL * EPS * H)   # 0.4096

_ctr = [0]


def _legalize_waits(nc):
    """This walrus build accepts 1 sync wait per instruction (2 on
    EventSemaphore); split excess waits into standalone waits."""
    def fix_block(blk):
        insts = list(blk.instructions)
        out = []
        changed = False
        for inst in insts:
            si = inst.sync_info
            waits = list(si.on_wait) if si and si.on_wait else []
            cap = 2 if isinstance(inst, mybir.InstEventSemaphore) else 1
            if len(waits) > cap:
                changed = True
                keep = waits[:cap]
                extra = waits[cap:]
                for i in range(0, len(extra), 2):
                    chunk = extra[i:i + 2]
                    _ctr[0] += 1
                    ev = mybir.InstEventSemaphore(
                        name=f"I-waitfix-{_ctr[0]}",
                        engine=inst.engine,
                        ins=[],
                        outs=[],
                        sync_info=mybir.SyncInfo(on_wait=chunk, on_update=[]),
                    )
                    out.append(ev)
                si.on_wait = keep
            out.append(inst)
        if changed:
            blk.instructions = out
        for sub in getattr(blk, "blocks", None) or []:
            fix_block(sub)

    for fn in nc.m.functions:
        for blk in fn.blocks:
            fix_block(blk)


def build_wprep():
    """Launch 1: conv [8,128,10,2,128] bf16 -> summed piece [8,128,128]."""
    nc = bass.Bass('TRN2', target_bir_lowering=False, debug=False)
    cw = nc.dram_tensor("cw", [NCH, P, NL // 2, 2, P], dt.bfloat16, kind="ExternalInput")
    piece = nc.dram_tensor("piece", [NCH, P, P], dt.bfloat16, kind="ExternalOutput")
    with TileContext(nc) as tc:
        with (
            tc.tile_pool(name="cwp", bufs=3) as cwp,
            tc.tile_pool(name="acc", bufs=2) as accp,
            tc.tile_pool(name="pc", bufs=4) as pcp,
        ):
            cw_sb = []
            for hc in range(NCH):
                t = cwp.tile([P, NL // 2, 2, P], dt.bfloat16, tag="cw",
                             name=f"cw{hc}")
                nc.sync.dma_start(t[:], cw[hc])
                cw_sb.append(t)
            for hc in range(NCH):
                t = cw_sb[hc]
                eng = nc.vector if hc % 2 == 0 else nc.gpsimd
                a = accp.tile([P, 5, 2, P], dt.float32, tag=f"a{hc % 2}",
                              name=f"a{hc}")
                b = accp.tile([P, 5, P], dt.float32, tag=f"b{hc % 2}",
                              name=f"b{hc}")
                c = accp.tile([P, 2, P], dt.float32, tag=f"c{hc % 2}",
                              name=f"c{hc}")
                d = accp.tile([P, P], dt.float32, tag=f"d{hc % 2}",
                              name=f"d{hc}")
                eng.tensor_add(a[:], t[:, 0:5, :, :], t[:, 5:10, :, :])
                eng.tensor_add(b[:], a[:, :, 0, :], a[:, :, 1, :])
                eng.tensor_add(c[:], b[:, 0:2, :], b[:, 2:4, :])
                eng.tensor_add(d[:], c[:, 0, :], c[:, 1, :])
                pc = pcp.tile([P, P], dt.bfloat16, tag="piece", name=f"pc{hc}")
                eng.tensor_add(pc[:], d[:], b[:, 4, :])
                nc.scalar.dma_start(piece[hc], pc[:])
    _legalize_waits(nc)
    return nc


def build_gemm():
    """Launch 2: xh [4,128,8,256] bf16 @ wt [128,8,8,128] bf16 + RMSNorm."""
    nc = bass.Bass('TRN2', target_bir_lowering=False, debug=False)
    xh = nc.dram_tensor("xh", [NTG, P, NCH, TGW], dt.bfloat16, kind="ExternalInput")
    wt = nc.dram_tensor("wt", [P, NCH, NCORES, P], dt.bfloat16, kind="ExternalInput")
    nw = nc.dram_tensor("nw", [H], dt.float32, kind="ExternalInput")
    y = nc.dram_tensor("y", [TPC, H], dt.bfloat16, kind="ExternalOutput")
    mult = mybir.AluOpType.mult
    with TileContext(nc) as tc:
        with (
            tc.tile_pool(name="w", bufs=1) as wp,
            tc.tile_pool(name="sq", bufs=2) as sqp,
            tc.tile_pool(name="stat", bufs=8) as stat,
            tc.tile_pool(name="y", bufs=4) as yp,
            tc.tile_pool(name="psum", bufs=4, space="PSUM") as psum,
        ):
            wt_sb = wp.tile([P, NCH, NCORES, P], dt.bfloat16, tag="wt_sb")
            xh_sb = wp.tile([P, NTG, NCH, TGW], dt.bfloat16, tag="xh_sb")
            nwb = wp.tile([P, H], dt.float32, tag="nwb")
            wm = wp.tile([P, 512], dt.bfloat16, tag="wm")

            # PE warm-up while weights/tokens stream in (cold PE = 1.2 GHz)
            nc.gpsimd.memset(wm[:], 0.0)
            wu = psum.tile([P, H], dt.float32, tag="pt", name="wu")
            for i in range(10):
                nc.tensor.matmul(wu[:, 0:512], wm[:, 0:P], wm[:],
                                 start=True, stop=True)

            nc.scalar.dma_start(nwb[:], nw[None, :].partition_broadcast(P))
            # wt split across both HWDGE queues, x behind it
            nc.sync.dma_start(wt_sb[:, 0:NCH // 2], wt[:, 0:NCH // 2])
            nc.scalar.dma_start(wt_sb[:, NCH // 2:], wt[:, NCH // 2:])
            for tg in range(NTG):
                q = nc.sync if tg % 2 == 0 else nc.scalar
                q.dma_start(xh_sb[:, tg], xh[tg])

            def norm_tile(tt, pt):
                # PSUM -> SBUF copy rounds to bf16 (= output precision) and
                # frees the PSUM banks for the next wave right away
                yc = yp.tile([P, H], dt.bfloat16, tag="yc", name=f"yc{tt}")
                nc.vector.tensor_copy(yc[:], pt[:])
                sq = sqp.tile([P, H], dt.bfloat16, tag="sq", name=f"sq{tt}")
                vb = stat.tile([P, 1], dt.float32, tag="vb", name=f"vb{tt}")
                nc.vector.scalar_tensor_tensor(
                    sq[:], yc[:], 1.0, yc[:], op0=mult, op1=mult,
                    accum_out=vb[:],
                )
                nc.vector.tensor_scalar(
                    vb[:], vb[:], SSQ_BIAS, None, mybir.AluOpType.add,
                )
                rv = stat.tile([P, 1], dt.float32, tag="rv", name=f"rv{tt}")
                nc.vector.reciprocal(rv[:], vb[:])
                s = stat.tile([P, 1], dt.float32, tag="s", name=f"s{tt}")
                nc.scalar.activation(
                    s[:], rv[:], mybir.ActivationFunctionType.Sqrt,
                    scale=float(H),
                )
                ysb = yp.tile([P, H], dt.bfloat16, tag="ysb", name=f"ysb{tt}")
                nc.vector.scalar_tensor_tensor(
                    ysb[:], yc[:], s[:], nwb[:], op0=mult, op1=mult,
                )
                nc.scalar.dma_start(y[tt * P:(tt + 1) * P, :], ysb[:])

            for tt in range(NT):
                pt = psum.tile([P, H], dt.float32, tag="pt", name=f"pt{tt}")
                tg, th = tt // 2, (tt % 2) * P
                lhsTs = xh_sb[:, tg, :, th:th + P]
                for hc in range(NCH):
                    for oh in range(2):
                        nc.tensor.matmul(
                            pt[:, oh * 512:(oh + 1) * 512],
                            lhsTs[:, hc],
                            wt_sb[:, hc, 4 * oh:4 * oh + 4, :],
                            start=(hc == 0), stop=(hc == NCH - 1),
                        )
                norm_tile(tt, pt)
    _legalize_waits(nc)
    return nc


_CACHE = {}


def _get(name, builder):
    if name not in _CACHE:
        _CACHE[name] = builder()
    return _CACHE[name]


def make_wprep_inputs(conv_w):
    """[20,1024,1024] f32 -> per-core [8 hc,128 h,10,2,128 o] bf16."""
    bf16 = ml_dtypes.bfloat16
    conv_w = np.asarray(conv_w, dtype=np.float32)
    in_maps = []
    for c in range(NCORES):
        a = conv_w[:, c * P:(c + 1) * P, :]          # [20 l, 128 o, 1024 h]
        cwc = np.ascontiguousarray(
            a.reshape(NL // 2, 2, P, NCH, P).transpose(3, 4, 0, 1, 2).astype(bf16)
        )
        in_maps.append({"cw": cwc})
    return in_maps


def assemble_wt(pieces):
    """8 x [8 hc,128 h,128 o_c] bf16 -> [128 p,8 hc,8 c,128 o] bf16."""
    # pieces[c][hc, p, o]; target wt[p, hc, c, o]
    stacked = np.stack(pieces, axis=0)               # [c, hc, p, o]
    return np.ascontiguousarray(stacked.transpose(2, 1, 0, 3))


def make_gemm_inputs(hidden_states, wt_host, norm_w):
    bf16 = ml_dtypes.bfloat16
    x = np.asarray(hidden_states, dtype=np.float32).reshape(TOK, H)
    norm_w = np.ascontiguousarray(np.asarray(norm_w, dtype=np.float32))
    in_maps = []
    for c in range(NCORES):
        xc = x[c * TPC:(c + 1) * TPC]                # [1024 t, 1024 h]
        xhc = np.ascontiguousarray(
            xc.reshape(NTG, TGW, NCH, P).transpose(0, 3, 2, 1).astype(bf16)
        )
        in_maps.append({"xh": xhc, "wt": wt_host, "nw": norm_w})
    return in_maps


def kernel(hidden_states, conv_w, norm_w):
    in_dtype = np.asarray(hidden_states).dtype
    core_ids = list(range(NCORES))

    nc1 = _get("wprep", build_wprep)
    res1 = bass_utils.run_bass_kernel_spmd(nc1, make_wprep_inputs(conv_w), core_ids)
    wt_host = assemble_wt([res1.results[i]["piece"] for i in range(NCORES)])

    nc2 = _get("gemm", build_gemm)
    res2 = bass_utils.run_bass_kernel_spmd(
        nc2, make_gemm_inputs(hidden_states, wt_host, norm_w), core_ids)
    ys = [res2.results[i]["y"].astype(np.float32) for i in range(NCORES)]
    return np.concatenate(ys, axis=0).reshape(B, S, H).astype(in_dtype, copy=False)



# revision 13
# speedup vs baseline: 1.1809x; 1.1809x over previous
"""Trainium2 Bass kernel for nn_Conv2DLinalgRMSNorm (two launches, bf16).

Math: out = RMSNormEps(x @ (sum_l conv_w[l])^T / 20) * norm_w
  where RMSNormEps(v) = v / sqrt(sum_h v^2 + eps*H) * sqrt(H)
The 1/20 folds into the norm bias: with y = x @ Wsum^T,
  out = y * sqrt(H) / sqrt(sum y^2 + NL^2*eps*H) * norm_w.

Strategy (8 NeuronCores):
  Host does dtype conversion / transposition / piece assembly only
  (all arithmetic stays on device).
  Launch 1 (weight prep): core c owns output-channel rows [128c,128c+128)
    of the 20 conv weights, laid out [8 hc][128 h][20 l][128 o] bf16.
    Per chunk a 5-level pairwise bf16 add tree on DVE (2x_1P mode) sums
    the 20 layers into piece[:, hc, :]. Reduction runs ONLY on DVE:
    GpSimd shares DVE's SBUF port (exclusive lock) so splitting across
    both self-contends, and tensor_reduce only runs in 1x mode.
    DMA: one chunk per transfer on a single ring (sync), chunk-ordered so
    DVE starts on chunk 0 early; one piece write at the end on the same
    ring. Only 2 engines are used, which keeps the semaphore count (and
    the per-semaphore epilogue-reset stream) small.
  Launch 2 (token-parallel GEMM + norm): core c takes 1024 tokens.
    Weights/tokens arrive as 16 SEPARATE SBUF tiles (8 wt chunks,
    8 xh half-groups) so Tile's whole-tile dependency tracking lets the
    first matmuls fire as soon as chunk 0 lands (~3us) instead of after
    the full weight matrix. Tiles 0-3 run hc-outer (consume chunks as
    they arrive); tiles 4-7 run tile-outer so the last PSUM tiles finish
    staggered and the norm chains pipeline. PE warm-up matmuls (on a
    DVE-memset tile) cover the DMA head so HAM reaches 2.4 GHz as real
    work starts. Norm per tile avoids 1x-mode DVE ops:
      ACT Copy pt->yc (bf16, frees PSUM early, ScalarE otherwise idle)
      DVE tensor_tensor_reduce: vb = eps-bias + sum(yc*yc)  (2x bf16)
      DVE reciprocal, ACT Sqrt(H * rv)  (only Copy+Sqrt on ACT)
      DVE tensor_scalar_mul yt = yc*s (4x), tensor_tensor ysb = yt*nw (2x)
    y written bf16 on the sync ring (drained by then), upcast on host.
"""
import numpy as np
import ml_dtypes

import concourse.bass as bass
import concourse.mybir as mybir
from concourse.tile import TileContext
from concourse import bass_utils

dt = mybir.dt
P = 128
H = 1024
NL = 20
B, S = 2, 4096
TOK = B * S            # 8192
NCORES = 8
TPC = TOK // NCORES    # 1024 tokens per core
NCH = 8                # h chunks of 128
NTG = 4                # token groups of 256
TGW = TPC // NTG       # 256
NT = TPC // P          # 8 token tiles
EPS = 1e-6
SSQ_BIAS = float(NL * NL * EPS * H)   # 0.4096

_ctr = [0]


def _legalize_waits(nc):
    """This walrus build accepts 1 sync wait per instruction (2 on
    EventSemaphore); split excess waits into standalone waits."""
    def fix_block(blk):
        insts = list(blk.instructions)
        out = []
        changed = False
        for inst in insts:
            si = inst.sync_info
            waits = list(si.on_wait) if si and si.on_wait else []
            cap = 2 if isinstance(inst, mybir.InstEventSemaphore) else 1
            if len(waits) > cap:
                changed = True
                keep = waits[:cap]
                extra = waits[cap:]
                for i in range(0, len(extra), 2):
                    chunk = extra[i:i + 2]
                    _ctr[0] += 1
                    ev = mybir.InstEventSemaphore(
                        name=f"I-waitfix-{_ctr[0]}",
                        engine=inst.engine,
                        ins=[],
                        outs=[],
                        sync_info=mybir.SyncInfo(on_wait=chunk, on_update=[]),
                    )
                    out.append(ev)
                si.on_wait = keep
            out.append(inst)
        if changed:
            blk.instructions = out
        for sub in getattr(blk, "blocks", None) or []:
            fix_block(sub)

    for fn in nc.m.functions:
        for blk in fn.blocks:
            fix_block(blk)


def build_wprep():
    """Launch 1: cw [8 hc,128 h,20 l,128 o] bf16 -> piece [128,8,128] bf16."""
    nc = bass.Bass('TRN2', target_bir_lowering=False, debug=False)
    cw = nc.dram_tensor("cw", [NCH, P, NL, P], dt.bfloat16, kind="ExternalInput")
    piece = nc.dram_tensor("piece", [P, NCH, P], dt.bfloat16, kind="ExternalOutput")
    NHALF = 2          # chunks 0..NHALF-1 split into o-halves for early start
    with TileContext(nc) as tc:
        with (
            tc.tile_pool(name="cwp", bufs=NCH + NHALF) as cwp,
            tc.tile_pool(name="up", bufs=2) as up,
            tc.tile_pool(name="pc", bufs=1) as pcp,
        ):
            pall = pcp.tile([P, NCH, P], dt.bfloat16, tag="pall")
            # tiny transfer to absorb the DMA-ring cold-start ramp; lands in
            # a pall slice that the hc0 adds overwrite later
            nc.sync.dma_start(pall[:, 0, :], cw[0, :, 0])

            tiles = []
            for hc in range(NHALF):
                h = [cwp.tile([P, NL, P // 2], dt.bfloat16, tag=f"cwh{hc}_{j}",
                              name=f"cwh{hc}_{j}") for j in range(2)]
                nc.sync.dma_start(h[0][:], cw[hc, :, :, 0:P // 2])
                nc.sync.dma_start(h[1][:], cw[hc, :, :, P // 2:P])
                tiles.append(h)
            for hc in range(NHALF, NCH):
                t = cwp.tile([P, NL, P], dt.bfloat16, tag="cw", name=f"cw{hc}")
                nc.sync.dma_start(t[:], cw[hc])
                tiles.append(t)
            for hc in range(NCH):
                if hc < NHALF:
                    # half-split chunk: level-1 adds per half, rest on a
                    # [2, n, 64] layout ending in the contiguous piece slice
                    u1 = up.tile([P, 2, 10, P // 2], dt.bfloat16, tag="u1h",
                                 name=f"u1h_{hc}")
                    for j in range(2):
                        nc.vector.tensor_add(u1[:, j], tiles[hc][j][:, 0:10],
                                             tiles[hc][j][:, 10:20])
                    u2 = up.tile([P, 2, 5, P // 2], dt.bfloat16, tag="u2h",
                                 name=f"u2h_{hc}")
                    nc.vector.tensor_add(u2[:], u1[:, :, 0:5], u1[:, :, 5:10])
                    u3 = up.tile([P, 2, 2, P // 2], dt.bfloat16, tag="u3h",
                                 name=f"u3h_{hc}")
                    nc.vector.tensor_add(u3[:], u2[:, :, 0:2], u2[:, :, 2:4])
                    u4 = up.tile([P, 2, P // 2], dt.bfloat16, tag="u4h",
                                 name=f"u4h_{hc}")
                    nc.vector.tensor_add(u4[:], u3[:, :, 0], u3[:, :, 1])
                    for j in range(2):
                        nc.vector.tensor_add(
                            pall[:, hc, j * (P // 2):(j + 1) * (P // 2)],
                            u4[:, j], u2[:, j, 4])
                    continue
                t = tiles[hc]
                u1 = up.tile([P, 10, P], dt.bfloat16, tag="u1", name=f"u1_{hc}")
                nc.vector.tensor_add(u1[:], t[:, 0:10], t[:, 10:20])
                u2 = up.tile([P, 5, P], dt.bfloat16, tag="u2", name=f"u2_{hc}")
                nc.vector.tensor_add(u2[:], u1[:, 0:5], u1[:, 5:10])
                u3 = up.tile([P, 2, P], dt.bfloat16, tag="u3", name=f"u3_{hc}")
                nc.vector.tensor_add(u3[:], u2[:, 0:2], u2[:, 2:4])
                u4 = up.tile([P, P], dt.bfloat16, tag="u4", name=f"u4_{hc}")
                nc.vector.tensor_add(u4[:], u3[:, 0], u3[:, 1])
                nc.vector.tensor_add(pall[:, hc, :], u4[:], u2[:, 4])
            nc.sync.dma_start(piece[:], pall[:])
    _legalize_waits(nc)
    return nc


def build_gemm():
    """Launch 2: xh [4,128,8,256] bf16 @ wt [128,8,8,128] bf16 + RMSNorm."""
    nc = bass.Bass('TRN2', target_bir_lowering=False, debug=False)
    xh = nc.dram_tensor("xh", [NTG, P, NCH, TGW], dt.bfloat16, kind="ExternalInput")
    wt = nc.dram_tensor("wt", [P, NCH, NCORES, P], dt.bfloat16, kind="ExternalInput")
    nw = nc.dram_tensor("nw", [H], dt.bfloat16, kind="ExternalInput")
    y = nc.dram_tensor("y", [TPC, H], dt.bfloat16, kind="ExternalOutput")
    mult = mybir.AluOpType.mult
    with TileContext(nc) as tc:
        with (
            tc.tile_pool(name="w", bufs=1) as wp,
            tc.tile_pool(name="sq", bufs=2) as sqp,
            tc.tile_pool(name="stat", bufs=8) as stat,
            tc.tile_pool(name="y", bufs=3) as yp,
            tc.tile_pool(name="psum", bufs=4, space="PSUM") as psum,
        ):
            nwb = wp.tile([P, H], dt.bfloat16, tag="nwb")
            wm = wp.tile([P, 512], dt.bfloat16, tag="wm")

            # PE warm-up while the first chunks stream in (cold PE = 1.2 GHz)
            nc.vector.memset(wm[:], 0.0)
            wu = psum.tile([P, H], dt.float32, tag="pt", name="wu")
            for i in range(5):
                nc.tensor.matmul(wu[:, 0:512], wm[:, 0:P], wm[:],
                                 start=True, stop=True)

            # separate SBUF tiles per chunk so matmuls depend only on the
            # chunk they read; single input ring (sync), priority order.
            # hc=0 is extra-fine (wt o-halves, per-tg hc0 x slices) so the
            # first matmuls fire while the DMA ring is still ramping up.
            wtc0 = [wp.tile([P, NCORES // 2, P], dt.bfloat16, tag=f"wt0{j}",
                            name=f"wtc0{j}") for j in range(2)]
            wtc = [None] + [wp.tile([P, NCORES, P], dt.bfloat16, tag=f"wt{hc}",
                                    name=f"wtc{hc}")
                            for hc in range(1, NCH)]
            xh0 = [wp.tile([P, TGW], dt.bfloat16, tag=f"xh0_{tg}",
                           name=f"xh0_{tg}") for tg in range(2)]
            x13 = [wp.tile([P, 3, TGW], dt.bfloat16, tag=f"x13_{tg}",
                           name=f"x13_{tg}") for tg in range(2)]
            xhh = {(tg, hf): wp.tile([P, NCH // 2, TGW], dt.bfloat16,
                                     tag=f"xh{tg}_{hf}", name=f"xhh{tg}_{hf}")
                   for tg in range(NTG) for hf in range(2)
                   if tg >= 2 or hf == 1}

            def dma_wt(hc):
                nc.sync.dma_start(wtc[hc][:], wt[:, hc])

            def dma_xh(tg, hf):
                nc.sync.dma_start(xhh[tg, hf][:],
                                  xh[tg, :, hf * (NCH // 2):(hf + 1) * (NCH // 2)])

            # ring-warming dummy into nwb (fully overwritten by the real
            # nwb broadcast on the scalar ring before any norm reads it)
            nc.sync.dma_start(nwb[:, 0:P], wt[:, 0, 0, :])
            nc.sync.dma_start(wtc0[0][:], wt[:, 0, 0:4])
            nc.sync.dma_start(xh0[0][:], xh[0, :, 0])
            nc.sync.dma_start(wtc0[1][:], wt[:, 0, 4:8])
            nc.sync.dma_start(xh0[1][:], xh[1, :, 0])
            dma_wt(1)
            nc.sync.dma_start(x13[0][:], xh[0, :, 1:4])
            nc.sync.dma_start(x13[1][:], xh[1, :, 1:4])
            dma_wt(2)
            dma_wt(3)
            dma_xh(0, 1)
            dma_xh(1, 1)
            dma_wt(4)
            dma_wt(5)
            dma_xh(2, 0)
            dma_xh(3, 0)
            dma_wt(6)
            dma_wt(7)
            dma_xh(2, 1)
            dma_xh(3, 1)
            nc.scalar.dma_start(nwb[:], nw[None, :].partition_broadcast(P))

            def norm_tile(tt, pt):
                # ACT evicts PSUM (bf16 round = output precision, frees the
                # banks); DVE ops then all run on bf16 SBUF in 2x/4x modes
                yc = yp.tile([P, H], dt.bfloat16, tag="yc", name=f"yc{tt}")
                nc.scalar.activation(yc[:], pt[:],
                                     mybir.ActivationFunctionType.Copy)
                sq = sqp.tile([P, H], dt.bfloat16, tag="sq", name=f"sq{tt}")
                vb = stat.tile([P, 1], dt.float32, tag="vb", name=f"vb{tt}")
                nc.vector.scalar_tensor_tensor(
                    sq[:], yc[:], 1.0, yc[:], op0=mult, op1=mult,
                    accum_out=vb[:],
                )
                nc.vector.tensor_scalar(
                    vb[:], vb[:], SSQ_BIAS, None, mybir.AluOpType.add,
                )
                rv = stat.tile([P, 1], dt.float32, tag="rv", name=f"rv{tt}")
                nc.vector.reciprocal(rv[:], vb[:])
                s = stat.tile([P, 1], dt.float32, tag="s", name=f"s{tt}")
                nc.scalar.activation(
                    s[:], rv[:], mybir.ActivationFunctionType.Sqrt,
                    scale=float(H),
                )
                yt = yp.tile([P, H], dt.bfloat16, tag="yt", name=f"yt{tt}")
                nc.vector.tensor_scalar_mul(yt[:], yc[:], s[:])
                ysb = yp.tile([P, H], dt.bfloat16, tag="ysb", name=f"ysb{tt}")
                nc.vector.tensor_tensor(ysb[:], yt[:], nwb[:], mult)
                nc.sync.dma_start(y[tt * P:(tt + 1) * P, :], ysb[:])

            def mm_tile(pt, tt, hc):
                tg, th = tt // 2, (tt % 2) * P
                if tg < 2 and hc == 0:
                    lhsT = xh0[tg][:, th:th + P]
                elif tg < 2 and hc < 4:
                    lhsT = x13[tg][:, hc - 1, th:th + P]
                else:
                    lhsT = xhh[tg, hc // 4][:, hc % 4, th:th + P]
                for oh in range(2):
                    rhs = (wtc0[oh][:] if hc == 0
                           else wtc[hc][:, 4 * oh:4 * oh + 4, :])
                    nc.tensor.matmul(
                        pt[:, oh * 512:(oh + 1) * 512],
                        lhsT, rhs,
                        start=(hc == 0), stop=(hc == NCH - 1),
                    )

            # group 0 (tiles 0-3): hc-outer — start on the first chunk
            pts0 = [psum.tile([P, H], dt.float32, tag="pt", name=f"pt{tt}")
                    for tt in range(4)]
            for hc in range(NCH):
                for tt in range(4):
                    mm_tile(pts0[tt], tt, hc)
            for tt in range(4):
                norm_tile(tt, pts0[tt])

            # group 1 (tiles 4-7): tile-outer — weights resident, staggered
            for tt in range(4, NT):
                pt = psum.tile([P, H], dt.float32, tag="pt", name=f"pt{tt}")
                for hc in range(NCH):
                    mm_tile(pt, tt, hc)
                norm_tile(tt, pt)
    _legalize_waits(nc)
    return nc


_CACHE = {}


def _get(name, builder):
    if name not in _CACHE:
        _CACHE[name] = builder()
    return _CACHE[name]


def make_wprep_inputs(conv_w):
    """[20,1024,1024] f32 -> per-core [8 hc,128 h,20 l,128 o] bf16."""
    bf16 = ml_dtypes.bfloat16
    conv_w = np.asarray(conv_w, dtype=np.float32)
    in_maps = []
    for c in range(NCORES):
        a = conv_w[:, c * P:(c + 1) * P, :]          # [20 l, 128 o, 1024 h]
        r = a.reshape(NL, P, NCH, P)                 # [l, o, hc, h]
        cwc = r.transpose(2, 3, 0, 1)                # [hc, h, l, o]
        in_maps.append({"cw": np.ascontiguousarray(cwc.astype(bf16))})
    return in_maps


def assemble_wt(pieces):
    """8 x [128 h,8 hc,128 o_c] -> [128 p(h),8 hc,8 c,128 o] bf16."""
    stacked = np.stack(pieces, axis=2)               # [h, hc, c, o]
    return np.ascontiguousarray(stacked.astype(ml_dtypes.bfloat16))


def make_gemm_inputs(hidden_states, wt_host, norm_w):
    bf16 = ml_dtypes.bfloat16
    x = np.asarray(hidden_states, dtype=np.float32).reshape(TOK, H)
    nw = np.ascontiguousarray(np.asarray(norm_w, dtype=np.float32).astype(bf16))
    in_maps = []
    for c in range(NCORES):
        xc = x[c * TPC:(c + 1) * TPC]                # [1024 t, 1024 h]
        xhc = np.ascontiguousarray(
            xc.reshape(NTG, TGW, NCH, P).transpose(0, 3, 2, 1).astype(bf16)
        )
        in_maps.append({"xh": xhc, "wt": wt_host, "nw": nw})
    return in_maps


def kernel(hidden_states, conv_w, norm_w):
    in_dtype = np.asarray(hidden_states).dtype
    core_ids = list(range(NCORES))

    nc1 = _get("wprep", build_wprep)
    res1 = bass_utils.run_bass_kernel_spmd(nc1, make_wprep_inputs(conv_w), core_ids)
    wt_host = assemble_wt([res1.results[i]["piece"] for i in range(NCORES)])

    nc2 = _get("gemm", build_gemm)
    res2 = bass_utils.run_bass_kernel_spmd(
        nc2, make_gemm_inputs(hidden_states, wt_host, norm_w), core_ids)
    ys = [res2.results[i]["y"].astype(np.float32) for i in range(NCORES)]
    return np.concatenate(ys, axis=0).reshape(B, S, H).astype(in_dtype, copy=False)


# revision 17
# speedup vs baseline: 1.2159x; 1.0297x over previous
"""Trainium2 Bass kernel for nn_Conv2DLinalgRMSNorm (two launches, bf16).

Math: out = RMSNormEps(x @ (sum_l conv_w[l])^T / 20) * norm_w
  where RMSNormEps(v) = v / sqrt(sum_h v^2 + eps*H) * sqrt(H)
The 1/20 folds into the norm bias: with y = x @ Wsum^T,
  out = y * sqrt(H) / sqrt(sum y^2 + NL^2*eps*H) * norm_w.

Strategy (8 NeuronCores):
  Host does dtype conversion / transposition / piece assembly only
  (all arithmetic stays on device).
  Launch 1 (weight prep): core c owns output-channel rows [128c,128c+128)
    of the 20 conv weights, laid out [8 hc][128 h][20 l][128 o] bf16.
    Per chunk a 5-level pairwise bf16 add tree on DVE (2x_1P mode) sums
    the 20 layers into piece[:, hc, :]. Reduction runs ONLY on DVE:
    GpSimd shares DVE's SBUF port (exclusive lock) so splitting across
    both self-contends, and tensor_reduce only runs in 1x mode.
    DMA: one chunk per transfer on a single ring (sync), chunk-ordered so
    DVE starts on chunk 0 early; one piece write at the end on the same
    ring. Only 2 engines are used, which keeps the semaphore count (and
    the per-semaphore epilogue-reset stream) small.
  Launch 2 (token-parallel GEMM + norm): core c takes 1024 tokens.
    Weights/tokens arrive as 16 SEPARATE SBUF tiles (8 wt chunks,
    8 xh half-groups) so Tile's whole-tile dependency tracking lets the
    first matmuls fire as soon as chunk 0 lands (~3us) instead of after
    the full weight matrix. Tiles 0-3 run hc-outer (consume chunks as
    they arrive); tiles 4-7 run tile-outer so the last PSUM tiles finish
    staggered and the norm chains pipeline. PE warm-up matmuls (on a
    DVE-memset tile) cover the DMA head so HAM reaches 2.4 GHz as real
    work starts. Norm per tile avoids 1x-mode DVE ops:
      ACT Copy pt->yc (bf16, frees PSUM early, ScalarE otherwise idle)
      DVE tensor_tensor_reduce: vb = eps-bias + sum(yc*yc)  (2x bf16)
      DVE reciprocal, ACT Sqrt(H * rv)  (only Copy+Sqrt on ACT)
      DVE tensor_scalar_mul yt = yc*s (4x), tensor_tensor ysb = yt*nw (2x)
    y written bf16 on the sync ring (drained by then), upcast on host.
"""
import numpy as np
import ml_dtypes

import concourse.bass as bass
import concourse.mybir as mybir
from concourse.tile import TileContext
from concourse import bass_utils

dt = mybir.dt
P = 128
H = 1024
NL = 20
B, S = 2, 4096
TOK = B * S            # 8192
NCORES = 8
TPC = TOK // NCORES    # 1024 tokens per core
NCH = 8                # h chunks of 128
NTG = 4                # token groups of 256
TGW = TPC // NTG       # 256
NT = TPC // P          # 8 token tiles
EPS = 1e-6
SSQ_BIAS = float(NL * NL * EPS * H)   # 0.4096
NHALF = 2          # L1 chunks pre-split into o-halves for early DVE start

_ctr = [0]


def _legalize_waits(nc):
    """This walrus build accepts 1 sync wait per instruction (2 on
    EventSemaphore); split excess waits into standalone waits."""
    def fix_block(blk):
        insts = list(blk.instructions)
        out = []
        changed = False
        for inst in insts:
            si = inst.sync_info
            waits = list(si.on_wait) if si and si.on_wait else []
            cap = 2 if isinstance(inst, mybir.InstEventSemaphore) else 1
            if len(waits) > cap:
                changed = True
                keep = waits[:cap]
                extra = waits[cap:]
                for i in range(0, len(extra), 2):
                    chunk = extra[i:i + 2]
                    _ctr[0] += 1
                    ev = mybir.InstEventSemaphore(
                        name=f"I-waitfix-{_ctr[0]}",
                        engine=inst.engine,
                        ins=[],
                        outs=[],
                        sync_info=mybir.SyncInfo(on_wait=chunk, on_update=[]),
                    )
                    out.append(ev)
                si.on_wait = keep
            out.append(inst)
        if changed:
            blk.instructions = out
        for sub in getattr(blk, "blocks", None) or []:
            fix_block(sub)

    for fn in nc.m.functions:
        for blk in fn.blocks:
            fix_block(blk)


def build_wprep():
    """Launch 1: cw [8 hc,128 h,20 l,128 o] bf16 -> piece [128,8,128] bf16."""
    nc = bass.Bass('TRN2', target_bir_lowering=False, debug=False)
    # chunks 0..1 pre-split into contiguous o-halves by the host
    cwh = nc.dram_tensor("cwh", [NHALF, 2, P, NL, P // 2], dt.bfloat16,
                         kind="ExternalInput")
    cw = nc.dram_tensor("cw", [NCH - NHALF, P, NL, P], dt.bfloat16,
                        kind="ExternalInput")
    piece = nc.dram_tensor("piece", [P, NCH, P], dt.bfloat16, kind="ExternalOutput")
    with TileContext(nc) as tc:
        with (
            tc.tile_pool(name="cwp", bufs=NCH + NHALF) as cwp,
            tc.tile_pool(name="up", bufs=2) as up,
            tc.tile_pool(name="pc", bufs=1) as pcp,
        ):
            pall = pcp.tile([P, NCH, P], dt.bfloat16, tag="pall")
            # tiny transfer to absorb the DMA-ring cold-start ramp; lands in
            # a pall slice that the hc0 adds overwrite later
            nc.sync.dma_start(pall[:, 0, :], cw[0, :, 0])

            tiles = []
            for hc in range(NHALF):
                h = [cwp.tile([P, NL, P // 2], dt.bfloat16, tag=f"cwh{hc}_{j}",
                              name=f"cwh{hc}_{j}") for j in range(2)]
                nc.sync.dma_start(h[0][:], cwh[hc, 0])
                nc.sync.dma_start(h[1][:], cwh[hc, 1])
                tiles.append(h)
            for hc in range(NHALF, NCH):
                t = cwp.tile([P, NL, P], dt.bfloat16, tag="cw", name=f"cw{hc}")
                nc.sync.dma_start(t[:], cw[hc - NHALF])
                tiles.append(t)
            for hc in range(NCH):
                if hc < NHALF:
                    # half-split chunk: level-1 adds per half, rest on a
                    # [2, n, 64] layout ending in the contiguous piece slice
                    u1 = up.tile([P, 2, 10, P // 2], dt.bfloat16, tag="u1h",
                                 name=f"u1h_{hc}")
                    for j in range(2):
                        nc.vector.tensor_add(u1[:, j], tiles[hc][j][:, 0:10],
                                             tiles[hc][j][:, 10:20])
                    u2 = up.tile([P, 2, 5, P // 2], dt.bfloat16, tag="u2h",
                                 name=f"u2h_{hc}")
                    nc.vector.tensor_add(u2[:], u1[:, :, 0:5], u1[:, :, 5:10])
                    u3 = up.tile([P, 2, 2, P // 2], dt.bfloat16, tag="u3h",
                                 name=f"u3h_{hc}")
                    nc.vector.tensor_add(u3[:], u2[:, :, 0:2], u2[:, :, 2:4])
                    u4 = up.tile([P, 2, P // 2], dt.bfloat16, tag="u4h",
                                 name=f"u4h_{hc}")
                    nc.vector.tensor_add(u4[:], u3[:, :, 0], u3[:, :, 1])
                    for j in range(2):
                        nc.vector.tensor_add(
                            pall[:, hc, j * (P // 2):(j + 1) * (P // 2)],
                            u4[:, j], u2[:, j, 4])
                    nc.scalar.dma_start(piece[:, hc, :], pall[:, hc, :])
                    continue
                t = tiles[hc]
                u1 = up.tile([P, 10, P], dt.bfloat16, tag="u1", name=f"u1_{hc}")
                nc.vector.tensor_add(u1[:], t[:, 0:10], t[:, 10:20])
                u2 = up.tile([P, 5, P], dt.bfloat16, tag="u2", name=f"u2_{hc}")
                nc.vector.tensor_add(u2[:], u1[:, 0:5], u1[:, 5:10])
                u3 = up.tile([P, 2, P], dt.bfloat16, tag="u3", name=f"u3_{hc}")
                nc.vector.tensor_add(u3[:], u2[:, 0:2], u2[:, 2:4])
                u4 = up.tile([P, P], dt.bfloat16, tag="u4", name=f"u4_{hc}")
                nc.vector.tensor_add(u4[:], u3[:, 0], u3[:, 1])
                nc.vector.tensor_add(pall[:, hc, :], u4[:], u2[:, 4])
                nc.scalar.dma_start(piece[:, hc, :], pall[:, hc, :])
    _legalize_waits(nc)
    return nc


def build_gemm():
    """Launch 2: xh [4,128,8,256] bf16 @ wt [128,8,8,128] bf16 + RMSNorm."""
    nc = bass.Bass('TRN2', target_bir_lowering=False, debug=False)
    xh = nc.dram_tensor("xh", [NTG, P, NCH, TGW], dt.bfloat16, kind="ExternalInput")
    wt = nc.dram_tensor("wt", [P, NCH, NCORES, P], dt.bfloat16, kind="ExternalInput")
    nw = nc.dram_tensor("nw", [H], dt.bfloat16, kind="ExternalInput")
    y = nc.dram_tensor("y", [TPC, H], dt.bfloat16, kind="ExternalOutput")
    mult = mybir.AluOpType.mult
    with TileContext(nc) as tc:
        with (
            tc.tile_pool(name="w", bufs=1) as wp,
            tc.tile_pool(name="sq", bufs=2) as sqp,
            tc.tile_pool(name="stat", bufs=8) as stat,
            tc.tile_pool(name="y", bufs=3) as yp,
            tc.tile_pool(name="psum", bufs=4, space="PSUM") as psum,
        ):
            nwb = wp.tile([P, H], dt.bfloat16, tag="nwb")
            wm = wp.tile([P, 512], dt.bfloat16, tag="wm")

            # PE warm-up while the first chunks stream in (cold PE = 1.2 GHz)
            nc.vector.memset(wm[:], 0.0)
            wu = psum.tile([P, H], dt.float32, tag="pt", name="wu")
            for i in range(7):
                nc.tensor.matmul(wu[:, 0:512], wm[:, 0:P], wm[:],
                                 start=True, stop=True)

            # separate SBUF tiles per chunk so matmuls depend only on the
            # chunk they read; single input ring (sync), priority order.
            # hc=0 is extra-fine (wt o-halves, per-tg hc0 x slices) so the
            # first matmuls fire while the DMA ring is still ramping up.
            wtc0 = [wp.tile([P, NCORES // 2, P], dt.bfloat16, tag=f"wt0{j}",
                            name=f"wtc0{j}") for j in range(2)]
            wtc = [None] + [wp.tile([P, NCORES, P], dt.bfloat16, tag=f"wt{hc}",
                                    name=f"wtc{hc}")
                            for hc in range(1, NCH)]
            xh0 = [wp.tile([P, TGW], dt.bfloat16, tag=f"xh0_{tg}",
                           name=f"xh0_{tg}") for tg in range(2)]
            x13 = [wp.tile([P, 3, TGW], dt.bfloat16, tag=f"x13_{tg}",
                           name=f"x13_{tg}") for tg in range(2)]
            xhh = {(tg, hf): wp.tile([P, NCH // 2, TGW], dt.bfloat16,
                                     tag=f"xh{tg}_{hf}", name=f"xhh{tg}_{hf}")
                   for tg in range(NTG) for hf in range(2)
                   if tg >= 2 or hf == 1}

            def dma_wt(hc):
                nc.sync.dma_start(wtc[hc][:], wt[:, hc])

            def dma_xh(tg, hf):
                nc.sync.dma_start(xhh[tg, hf][:],
                                  xh[tg, :, hf * (NCH // 2):(hf + 1) * (NCH // 2)])

            # ring-warming dummy into nwb (fully overwritten by the real
            # nwb broadcast on the scalar ring before any norm reads it)
            nc.sync.dma_start(nwb[:, 0:P], wt[:, 0, 0, :])
            nc.sync.dma_start(wtc0[0][:], wt[:, 0, 0:4])
            nc.sync.dma_start(xh0[0][:], xh[0, :, 0])
            nc.sync.dma_start(wtc0[1][:], wt[:, 0, 4:8])
            nc.sync.dma_start(xh0[1][:], xh[1, :, 0])
            dma_wt(1)
            nc.sync.dma_start(x13[0][:], xh[0, :, 1:4])
            nc.sync.dma_start(x13[1][:], xh[1, :, 1:4])
            dma_wt(2)
            dma_wt(3)
            dma_xh(0, 1)
            dma_xh(1, 1)
            dma_wt(4)
            dma_wt(5)
            dma_xh(2, 0)
            dma_xh(3, 0)
            dma_wt(6)
            dma_wt(7)
            dma_xh(2, 1)
            dma_xh(3, 1)
            nc.scalar.dma_start(nwb[:], nw[None, :].partition_broadcast(P))

            def norm_tile(tt, pt):
                # ACT evicts PSUM (bf16 round = output precision, frees the
                # banks); DVE ops then all run on bf16 SBUF in 2x/4x modes
                yc = yp.tile([P, H], dt.bfloat16, tag="yc", name=f"yc{tt}")
                nc.scalar.activation(yc[:], pt[:],
                                     mybir.ActivationFunctionType.Copy)
                sq = sqp.tile([P, H], dt.bfloat16, tag="sq", name=f"sq{tt}")
                vb = stat.tile([P, 1], dt.float32, tag="vb", name=f"vb{tt}")
                nc.vector.scalar_tensor_tensor(
                    sq[:], yc[:], 1.0, yc[:], op0=mult, op1=mult,
                    accum_out=vb[:],
                )
                nc.vector.tensor_scalar(
                    vb[:], vb[:], SSQ_BIAS, None, mybir.AluOpType.add,
                )
                rv = stat.tile([P, 1], dt.float32, tag="rv", name=f"rv{tt}")
                nc.vector.reciprocal(rv[:], vb[:])
                s = stat.tile([P, 1], dt.float32, tag="s", name=f"s{tt}")
                nc.scalar.activation(
                    s[:], rv[:], mybir.ActivationFunctionType.Sqrt,
                    scale=float(H),
                )
                yt = yp.tile([P, H], dt.bfloat16, tag="yt", name=f"yt{tt}")
                nc.vector.tensor_scalar_mul(yt[:], yc[:], s[:])
                ysb = yp.tile([P, H], dt.bfloat16, tag="ysb", name=f"ysb{tt}")
                nc.vector.tensor_tensor(ysb[:], yt[:], nwb[:], mult)
                nc.sync.dma_start(y[tt * P:(tt + 1) * P, :], ysb[:])

            def mm_tile(pt, tt, hc):
                tg, th = tt // 2, (tt % 2) * P
                if tg < 2 and hc == 0:
                    lhsT = xh0[tg][:, th:th + P]
                elif tg < 2 and hc < 4:
                    lhsT = x13[tg][:, hc - 1, th:th + P]
                else:
                    lhsT = xhh[tg, hc // 4][:, hc % 4, th:th + P]
                for oh in range(2):
                    rhs = (wtc0[oh][:] if hc == 0
                           else wtc[hc][:, 4 * oh:4 * oh + 4, :])
                    nc.tensor.matmul(
                        pt[:, oh * 512:(oh + 1) * 512],
                        lhsT, rhs,
                        start=(hc == 0), stop=(hc == NCH - 1),
                    )

            # group 0 (tiles 0-3): hc-outer — start on the first chunk
            pts0 = [psum.tile([P, H], dt.float32, tag="pt", name=f"pt{tt}")
                    for tt in range(4)]
            for hc in range(NCH):
                for tt in range(4):
                    mm_tile(pts0[tt], tt, hc)
            for tt in range(4):
                norm_tile(tt, pts0[tt])

            # group 1 (tiles 4-7): tile-outer — weights resident, staggered
            for tt in range(4, NT):
                pt = psum.tile([P, H], dt.float32, tag="pt", name=f"pt{tt}")
                for hc in range(NCH):
                    mm_tile(pt, tt, hc)
                norm_tile(tt, pt)
    _legalize_waits(nc)
    return nc


_CACHE = {}


def _get(name, builder):
    if name not in _CACHE:
        _CACHE[name] = builder()
    return _CACHE[name]


def make_wprep_inputs(conv_w):
    """[20,1024,1024] f32 -> per-core [hc,128 h,20 l,128 o] bf16 slabs,
    with chunks 0..NHALF-1 pre-split into contiguous o-halves."""
    bf16 = ml_dtypes.bfloat16
    conv_w = np.asarray(conv_w, dtype=np.float32)
    in_maps = []
    for c in range(NCORES):
        a = conv_w[:, c * P:(c + 1) * P, :]          # [20 l, 128 o, 1024 h]
        r = a.reshape(NL, P, NCH, P)                 # [l, o, hc, h]
        cwc = np.ascontiguousarray(
            r.transpose(2, 3, 0, 1).astype(bf16))    # [hc, h, l, o]
        halves = cwc[:NHALF].reshape(NHALF, P, NL, 2, P // 2)
        cwh = np.ascontiguousarray(halves.transpose(0, 3, 1, 2, 4))
        in_maps.append({"cwh": cwh,
                        "cw": np.ascontiguousarray(cwc[NHALF:])})
    return in_maps


def assemble_wt(pieces):
    """8 x [128 h,8 hc,128 o_c] -> [128 p(h),8 hc,8 c,128 o] bf16."""
    stacked = np.stack(pieces, axis=2)               # [h, hc, c, o]
    return np.ascontiguousarray(stacked.astype(ml_dtypes.bfloat16))


def make_gemm_inputs(hidden_states, wt_host, norm_w):
    bf16 = ml_dtypes.bfloat16
    x = np.asarray(hidden_states, dtype=np.float32).reshape(TOK, H)
    nw = np.ascontiguousarray(np.asarray(norm_w, dtype=np.float32).astype(bf16))
    in_maps = []
    for c in range(NCORES):
        xc = x[c * TPC:(c + 1) * TPC]                # [1024 t, 1024 h]
        xhc = np.ascontiguousarray(
            xc.reshape(NTG, TGW, NCH, P).transpose(0, 3, 2, 1).astype(bf16)
        )
        in_maps.append({"xh": xhc, "wt": wt_host, "nw": nw})
    return in_maps


def kernel(hidden_states, conv_w, norm_w):
    in_dtype = np.asarray(hidden_states).dtype
    core_ids = list(range(NCORES))

    nc1 = _get("wprep", build_wprep)
    res1 = bass_utils.run_bass_kernel_spmd(nc1, make_wprep_inputs(conv_w), core_ids)
    wt_host = assemble_wt([res1.results[i]["piece"] for i in range(NCORES)])

    nc2 = _get("gemm", build_gemm)
    res2 = bass_utils.run_bass_kernel_spmd(
        nc2, make_gemm_inputs(hidden_states, wt_host, norm_w), core_ids)
    ys = [res2.results[i]["y"].astype(np.float32) for i in range(NCORES)]
    return np.concatenate(ys, axis=0).reshape(B, S, H).astype(in_dtype, copy=False)
